# revision 1
# baseline (speedup 1.0000x reference)
"""Trainium2 Bass kernel for nn_LowRankSig_HigherOrder — v3.

Math (per example, T=2048, U=64, F=64 incl. time channel):
  Xa  = concat(time, X);  dx = diff(Xa);  ya[t] = Xa[t-1]-Xa[0]
  M_k = dx @ K_k ;  E_k = ecum(M_k) = ya @ K_k
  out = (Xa[T-1]-Xa[0])@K0                                   (s0)
      + sum_t M2*(E1 + M1/2)                                 (T1)
      + sum_t M5*(EA2 + R1a/2 + R1b/3)                       (T2)
            R1a = M4*E3, R1b = M4*M3/2, EA2 = ecum(R1a+R1b)
      + sum_t M9*(EB3 + Sa/2 + Sb/3 + Sc/4)                  (T3)
            Ra = M7*E6, Rb = M7*M6/2, EB2 = ecum(Ra+Rb)
            Sa = M8*EB2, Sb = M8*Ra/2, Sc = M8*Rb/3
            EB3 = ecum(Sa+Sb+Sc)

Design notes (measured costs on this HW):
  * float32r single-pass matmuls (1 cyc/row); PSUM-fused projection combos:
      Z1 = ya@K1+dx@(K1/2), W2 = E3+M3/2, W2b = E3/2+M3/6,
      W3 = E6+M6/2, W3b = E6/2+M6/6, W3c = E6/6+M6/24
    so R1a+R1b = M4*W2, R1a/2+R1b/3 = M4*W2b, Sa+Sb+Sc = Sa + M8*(M7*W3b),
    Sb/3+Sc/4 = M8*(M7*W3c).
  * ya/dx/d0 are precomputed on host (DMA is idle; frees DVE/Pool prep).
  * DVE fast modes are bf16-only on this HW (fp16 runs 1x): L2 uses bf16
    2x tensor_tensor; the deep L3 chain stays fp32 (1x costs the same as
    fp16 and is exact) with plain TT split between Pool and DVE.
  * M2/M5/M9 are consumed from PSUM by accumulating scalar_tensor_tensor.
  * walrus --enable-ldw-opt=true (monkeypatched) dedupes weight reloads.
  * Emission is phase-interleaved across the two pairs so every engine
    queue always has ready work from the other pair.

Sharding: pure data parallel, 4 examples per core, packed 2-per-128-partitions.
"""

import numpy as np

import concourse.bass as bass
import concourse.bass_utils as _bu
import concourse.mybir as mybir
import concourse.tile as tile
from concourse.bass_utils import run_bass_kernel_spmd
from bass_rust import ScopedClock


# ---- walrus flag patch: dedupe identical LDWEIGHTS (saves ~210ns/matmul) ----
_orig_run_command = _bu.run_command


def _patched_run_command(cmd, **kw):
    if isinstance(cmd, list):
        pass  # ldw-opt incompatible with fp16 separate InstLdweights
    return _orig_run_command(cmd, **kw)


_bu.run_command = _patched_run_command


def _patched_drain_and_barrier(self, tick_clock, wait_clock):
    """Split the final drain's sem waits across multiple drain instructions
    (this walrus build rejects >1-2 sync waits per instruction)."""
    drain_inst = self.nc.sync.drain()
    wait_clock.add_sem_waits(drain_inst.ins, ScopedClock({None: tick_clock.global_clock}))
    si = drain_inst.ins.sync_info
    if si is not None and si.on_wait and len(si.on_wait) > 1:
        waits = list(si.on_wait)
        ups = list(si.on_update or [])
        drain_inst.ins.sync_info = mybir.SyncInfo(on_wait=waits[:1], on_update=ups)
        for w in waits[1:]:
            d2 = self.nc.sync.drain()
            d2.ins.sync_info = mybir.SyncInfo(on_wait=[w], on_update=[])

    self.nc.all_engine_barrier()
    popped = self.nc._tile_sem_poison_stack.pop()
    assert popped is self._sem_poison
    self.nc.clear_and_free_semaphores(list(self.sems.allocated().values()))
    self.nc.all_engine_barrier()


tile.TileContext._drain_and_barrier = _patched_drain_and_barrier


def _sanitize_waits(nc, limit=1):
    """Move excess sem waits onto same-engine NOPs inserted just before."""
    import bass_rust

    counter = [0]
    for f in nc.m.functions:
        for blk in f.blocks:
            il = blk.instructions
            i = 0
            while i < len(il):
                inst = il[i]
                si = inst.sync_info
                waits = list(si.on_wait) if (si is not None and si.on_wait) else []
                if len(waits) > limit:
                    keep, extra = waits[:limit], waits[limit:]
                    inst.sync_info = mybir.SyncInfo(
                        on_wait=keep, on_update=list(si.on_update or [])
                    )
                    for j in range(0, len(extra), limit):
                        counter[0] += 1
                        nop = bass_rust.InstNoOp(
                            name=f"waitnop-{counter[0]}", ins=[], outs=[]
                        )
                        nop.engine = inst.engine
                        nop.sync_info = mybir.SyncInfo(
                            on_wait=extra[j : j + limit], on_update=[]
                        )
                        il.insert(i, nop)
                        i += 1
                i += 1
    return counter[0]


B, T, FX = 32, 2048, 63
U = 64
NCORES = 8
BL = B // NCORES
NPAIR = BL // 2

FP = mybir.dt.float32
FR = mybir.dt.float32r
FH = mybir.dt.float16
BF = mybir.dt.bfloat16
AluOp = mybir.AluOpType

(SK0, SK1, SK1H, SK2, SK3, SK3H, SK3S, SK4, SK5,
 SK6, SK6H, SK6S, SK6T, SK7, SK8, SK9) = range(16)
NSLICE = 16


class PairState:
    pass


def build_nc(sanitize=True):
    nc = bass.Bass("TRN2", target_bir_lowering=False, debug=False)
    ya_d = nc.dram_tensor("yap", [NPAIR, 128, T], FH, kind="ExternalInput")
    dx_d = nc.dram_tensor("dxp", [NPAIR, 128, T], FH, kind="ExternalInput")
    d0_d = nc.dram_tensor("d0p", [NPAIR, 128, 1], FH, kind="ExternalInput")
    kst_d = nc.dram_tensor("kst", [NSLICE, 128, 128], FH, kind="ExternalInput")
    out_d = nc.dram_tensor("out", [NPAIR, 128, 1], FP, kind="ExternalOutput")

    with tile.TileContext(nc) as tc:
        with (
            tc.tile_pool(name="pool", bufs=2) as pool,
            tc.tile_pool(name="psum", bufs=2, space="PSUM") as psum,
            tc.tile_pool(name="phold", bufs=2, space="PSUM") as phold,
            tc.tile_pool(name="kpool", bufs=1) as kpool,
        ):
            kst = kpool.tile([128, NSLICE * 128], FH, tag="kst", name="kst")
            nc.sync.dma_start(
                kst[:].rearrange("p (k m) -> p k m", k=NSLICE),
                kst_d.ap().rearrange("k f m -> f k m"),
            )
            zz = kpool.tile([128, T], BF, tag="zz", name="zz")
            nc.gpsimd.memset(zz[:], 0.0)

            def proj(pl, tag, name, terms, p):
                halves = []
                halves = []
                for h in range(2):
                    ps = pl.tile([128, 1024], FP, tag=tag, name=f"{name}ps{h}_{p}")
                    for j in range(2):
                        lo = h * 1024 + j * 512
                        for ti, (si, mv) in enumerate(terms):
                            nc.tensor.matmul(
                                ps[:, j * 512 : (j + 1) * 512],
                                kst[:, si * 128 : (si + 1) * 128],
                                mv[:, lo : lo + 512],
                                start=(ti == 0),
                                stop=(ti == len(terms) - 1),
                            )
                    halves.append(ps)
                return halves

            def copy_out(halves, name, dt, p, bufs=None, tag=None, eng=None):
                dst = pool.tile([128, T], dt, tag=f"c_{tag or name}",
                                name=f"{name}c{p}", bufs=bufs)
                for h in range(2):
                    if eng is None:
                        nc.scalar.copy(
                            out=dst[:, h * 1024 : (h + 1) * 1024],
                            in_=halves[h][:],
                        )
                    else:
                        eng.tensor_copy(
                            out=dst[:, h * 1024 : (h + 1) * 1024],
                            in_=halves[h][:],
                        )
                return dst

            def term(st, zt, mh, c0):
                for h in range(2):
                    scr = pool.tile([128, 1024], BF, tag="scr",
                                    name=f"scr{st.p}_{c0}_{h}", bufs=2)
                    nc.vector.scalar_tensor_tensor(
                        out=scr[:],
                        in0=zt[:, h * 1024 : (h + 1) * 1024],
                        scalar=1.0,
                        in1=mh[h][:],
                        op0=AluOp.mult,
                        op1=AluOp.mult,
                        accum_out=st.acc[:, c0 + h : c0 + h + 1],
                    )

            def big(name, dt, p, bufs=None):
                return pool.tile([128, T], dt, tag=name, name=f"{name}{p}",
                                 bufs=bufs)

            sts = []
            for p in range(NPAIR):
                st = PairState()
                st.p = p
                st.ya = pool.tile([128, T], FH, tag="ya", name=f"ya{p}")
                st.dx = pool.tile([128, T], FH, tag="dx", name=f"dx{p}")
                for h in range(2):
                    sl = slice(h * 1024, (h + 1) * 1024)
                    nc.sync.dma_start(st.dx[:, sl], dx_d[p, :, sl])
                    nc.sync.dma_start(st.ya[:, sl], ya_d[p, :, sl])
                st.d0 = pool.tile([128, 1], FH, tag="d0", name=f"d0{p}")
                nc.sync.dma_start(st.d0[:], d0_d[p])
                st.acc = kpool.tile([128, 8], FP, tag=f"acc{p}", name=f"acc{p}")

                sts.append(st)

            # ---- phase functions (emitted pair-staggered below) ----
            def ph_l1(st):
                p, ya, dx = st.p, st.ya, st.dx
                z1h = proj(psum, "ps", "z1", [(SK1, ya), (SK1H, dx)], p)
                st.z1c = copy_out(z1h, "z1", BF, p)
                st.m2h = proj(phold, "ph", "m2", [(SK2, dx)], p)
                term(st, st.z1c, st.m2h, 0)
                s0 = phold.tile([128, 1024], FP, tag="ph", name=f"s0ps{p}")
                nc.tensor.matmul(
                    s0[:, 0:1],
                    kst[:, SK0 * 128 : (SK0 + 1) * 128],
                    st.d0[:, 0:1],
                    start=True, stop=True,
                )
                st.s0sb = pool.tile([128, 1], FP, tag="s0sb", name=f"s0sb{p}")
                nc.vector.tensor_copy(out=st.s0sb[:], in_=s0[:, 0:1])

            def ph_l2a(st):
                p, ya, dx = st.p, st.ya, st.dx
                w2h = proj(psum, "ps", "w2", [(SK3, ya), (SK3H, dx)], p)
                st.w2c = copy_out(w2h, "w2", BF, p)
                m4h = proj(psum, "ps", "m4", [(SK4, dx)], p)
                st.m4c = copy_out(m4h, "m4", BF, p)
                w2bh = proj(psum, "ps", "w2b", [(SK3H, ya), (SK3S, dx)], p)
                st.w2bc = copy_out(w2bh, "w2b", BF, p)
                st.p2ab = big("p2ab", BF, p)
                nc.vector.tensor_tensor(
                    out=st.p2ab[:], in0=st.m4c[:], in1=st.w2c[:], op=AluOp.mult
                )
                st.ea2 = big("eax", FP, p, bufs=2)
                nc.gpsimd.memset(st.ea2[:, 0:1], 0.0)
                nc.vector.tensor_tensor_scan(
                    out=st.ea2[:, 1:T], data0=zz[:, 0 : T - 1],
                    data1=st.p2ab[:, 0 : T - 1],
                    initial=0.0, op0=AluOp.add, op1=AluOp.add,
                )

            def ph_l2b(st):
                p = st.p
                st.v2 = big("p2ab", BF, p)
                nc.vector.tensor_tensor(
                    out=st.v2[:], in0=st.m4c[:], in1=st.w2bc[:], op=AluOp.mult
                )
                st.z2 = big("z2", FP, p)
                nc.vector.scalar_tensor_tensor(
                    out=st.z2[:], in0=st.ea2[:], scalar=1.0, in1=st.v2[:],
                    op0=AluOp.mult, op1=AluOp.add,
                )
                st.m5h = proj(phold, "ph", "m5", [(SK5, st.dx)], st.p)
                term(st, st.z2, st.m5h, 2)

            def ph_l3a(st):
                p, ya, dx = st.p, st.ya, st.dx
                w3h = proj(psum, "ps", "w3", [(SK6, ya), (SK6H, dx)], p)
                st.w3c = copy_out(w3h, "w3", FP, p, bufs=1, tag="w3x")
                m7h = proj(psum, "ps", "m7", [(SK7, dx)], p)
                st.m7c = copy_out(m7h, "m7", BF, p, bufs=2)
                st.p3ab = big("l3x", FP, p, bufs=1)
                for h in range(2):
                    hs = slice(h * 1024, (h + 1) * 1024)
                    nc.vector.scalar_tensor_tensor(
                        out=st.p3ab[:, hs], in0=st.w3c[:, hs], scalar=1.0,
                        in1=m7h[h][:], op0=AluOp.mult, op1=AluOp.mult,
                    )
                st.eb2 = big("l3y", FP, p, bufs=1)
                nc.gpsimd.memset(st.eb2[:, 0:1], 0.0)
                nc.vector.tensor_tensor_scan(
                    out=st.eb2[:, 1:T], data0=zz[:, 0 : T - 1],
                    data1=st.p3ab[:, 0 : T - 1],
                    initial=0.0, op0=AluOp.add, op1=AluOp.add,
                )

            def ph_l3b(st):
                p, ya, dx = st.p, st.ya, st.dx
                w3bh = proj(psum, "ps", "w3b", [(SK6H, ya), (SK6S, dx)], p)
                st.w3bc = copy_out(w3bh, "w3b", BF, p, bufs=1, tag="w3x")
                m8h = proj(psum, "ps", "m8", [(SK8, dx)], p)
                st.m8c = copy_out(m8h, "m8", BF, p, bufs=2)
                st.v3b = big("l3z", BF, p, bufs=1)
                nc.vector.tensor_tensor(
                    out=st.v3b[:], in0=st.m7c[:], in1=st.w3bc[:], op=AluOp.mult
                )
                st.sa = big("sa", FP, p, bufs=1)
                for h in range(2):
                    hs = slice(h * 1024, (h + 1) * 1024)
                    nc.vector.scalar_tensor_tensor(
                        out=st.sa[:, hs], in0=st.eb2[:, hs], scalar=1.0,
                        in1=m8h[h][:], op0=AluOp.mult, op1=AluOp.mult,
                    )
                st.sbc = big("l3w", BF, p, bufs=1)
                nc.vector.tensor_tensor(
                    out=st.sbc[:], in0=st.m8c[:], in1=st.v3b[:], op=AluOp.mult
                )
                st.eb3 = big("eax", FP, p, bufs=2)
                nc.gpsimd.memset(st.eb3[:, 0:1], 0.0)
                nc.vector.tensor_tensor_scan(
                    out=st.eb3[:, 1:T], data0=st.sa[:, 0 : T - 1],
                    data1=st.sbc[:, 0 : T - 1],
                    initial=0.0, op0=AluOp.add, op1=AluOp.add,
                )

            def ph_l3c(st):
                p, ya, dx = st.p, st.ya, st.dx
                w3ch = proj(psum, "ps", "w3cv", [(SK6S, ya), (SK6T, dx)], p)
                st.w3cc = copy_out(w3ch, "w3cv", BF, p, bufs=1, tag="w3x")
                st.v3c = big("l3x", BF, p, bufs=1)
                nc.vector.tensor_tensor(
                    out=st.v3c[:], in0=st.m7c[:], in1=st.w3cc[:], op=AluOp.mult
                )
                st.q3c = big("l3y", BF, p, bufs=1)
                nc.vector.tensor_tensor(
                    out=st.q3c[:], in0=st.m8c[:], in1=st.v3c[:], op=AluOp.mult
                )
                st.z3a = big("l3z", FP, p, bufs=1)
                nc.vector.scalar_tensor_tensor(
                    out=st.z3a[:], in0=st.sa[:], scalar=0.5, in1=st.eb3[:],
                    op0=AluOp.mult, op1=AluOp.add,
                )
                st.z3b = big("l3w", FP, p, bufs=1)
                nc.vector.scalar_tensor_tensor(
                    out=st.z3b[:], in0=st.z3a[:], scalar=1.0, in1=st.q3c[:],
                    op0=AluOp.mult, op1=AluOp.add,
                )
                st.m9h = proj(phold, "ph", "m9", [(SK9, st.dx)], st.p)
                term(st, st.z3b, st.m9h, 4)

            def ph_fin(st):
                p = st.p
                red = pool.tile([128, 1], FP, tag="red", name=f"red{p}")
                nc.vector.tensor_reduce(
                    out=red[:], in_=st.acc[:, 0:6], axis=mybir.AxisListType.X,
                    op=AluOp.add,
                )
                outt = pool.tile([128, 1], FP, tag="outt", name=f"outt{p}")
                nc.vector.tensor_tensor(
                    out=outt[:], in0=red[:], in1=st.s0sb[:], op=AluOp.add
                )
                nc.sync.dma_start(out_d[p], outt[:])

            # pair-staggered emission: pair 1 trails pair 0 by one phase so
            # each engine queue always holds ready work from the other pair
            phases = [ph_l1, ph_l2a, ph_l2b, ph_l3a, ph_l3b, ph_l3c, ph_fin]
            sched = []
            for i in range(len(phases) + 1):
                if i < len(phases):
                    sched.append((phases[i], 0))
                if i >= 1:
                    sched.append((phases[i - 1], 1))
            for fn, p in sched:
                fn(sts[p])
    if sanitize:
        n = _sanitize_waits(nc)
        print(f"[kernel] split {n} excess sem waits onto NOPs")
    return nc


_CACHE = {}


def _get_nc():
    if "nc" not in _CACHE:
        _CACHE["nc"] = build_nc()
    return _CACHE["nc"]


def _bd(k64):
    b = np.zeros((128, 128), dtype=np.float32)
    b[:U, :U] = k64
    b[U:, U:] = k64
    return b


def _marshal(X, kernel):
    Xf = np.ascontiguousarray(X, dtype=np.float32)
    tch = np.arange(T, dtype=np.float32) * (2.0 / (T - 1.0)) - 1.0
    Xa = np.empty((B, T, U), dtype=np.float32)
    Xa[:, :, 0] = tch[None, :]
    Xa[:, :, 1:] = Xf

    ya = np.zeros_like(Xa)
    ya[:, 1:] = Xa[:, :-1] - Xa[:, :1]
    dxa = np.zeros_like(Xa)
    dxa[:, 1:] = Xa[:, 1:] - Xa[:, :-1]
    d0 = Xa[:, -1] - Xa[:, 0]  # [B, U]

    def pack(A):  # [B, T, U] -> [NCORES, NPAIR, 128, T]
        return np.ascontiguousarray(
            A.reshape(NCORES, NPAIR, 2, T, U).transpose(0, 1, 2, 4, 3)
        ).reshape(NCORES, NPAIR, 128, T)

    yap = pack(ya).astype(np.float16)
    dxp = pack(dxa).astype(np.float16)
    d0p = np.ascontiguousarray(d0.reshape(NCORES, NPAIR, 128, 1)).astype(np.float16)

    kf = np.asarray(kernel, dtype=np.float32)
    K = [kf[:, i, :] for i in range(10)]
    slices = [
        K[0], K[1], 0.5 * K[1], K[2],
        K[3], 0.5 * K[3], K[3] / 6.0, K[4], K[5],
        K[6], 0.5 * K[6], K[6] / 6.0, K[6] / 24.0, K[7], K[8], K[9],
    ]
    kst16 = np.stack([_bd(s) for s in slices]).astype(np.float16)
    return yap, dxp, d0p, kst16


def run(X, kernel, trace=False):
    nc = _get_nc()
    yap, dxp, d0p, kst = _marshal(X, kernel)
    in_maps = [
        {"yap": yap[c], "dxp": dxp[c], "d0p": d0p[c], "kst": kst}
        for c in range(NCORES)
    ]
    res = run_bass_kernel_spmd(nc, in_maps, list(range(NCORES)), trace=trace)
    out = np.stack([r["out"] for r in res.results])
    out = out.reshape(NCORES, NPAIR, 2, U).reshape(B, U)
    return out, res


def kernel(X, kernel):
    out, _ = run(X, kernel)
    return out



# revision 6
# speedup vs baseline: 1.0671x; 1.0671x over previous
"""Trainium2 Bass kernel for nn_LowRankSig_HigherOrder — v4.

Math (per example, T=2048, U=64, F=64 incl. time channel), with
summation-by-parts to eliminate the outer scans:
  dx[t] = Xa[t]-Xa[t-1]; ya[t] = Xa[t-1]-Xa[0]
  streams: yb = ya+dx/2, yc = ya/2+dx/6, yd2 = ya/3+dx/12,
           yr[t] = Xa[T-1]-Xa[t]
  projections (one matmul pass each unless noted):
    z1=yb@K1, m2=dx@K2, w2=yb@K3, w2b=yc@K3, m4=dx@K4,
    S5=yr@K5, m5=dx@K5, w3=yb@K6, w3b=yc@K6, w3c'=yd2@K6,
    m7=dx@K7, m8=dx@K8, m9h=dx@(K9/2), SA9=yr@K9+dx@(K9/2) (2-pass)
  out = d0@K0
      + Sum_t m2*z1                                   (T1)
      + Sum_t m4*(w2*S5 + w2b*m5)                     (T2)
      + Sum_t m8*((EB2+w)*SA9 + (v3c'-w)*m9h)         (T3)
        p = m7*w3; EB2 = ecum(p)  (the only scan)
        w = m7*w3b; v3c' = m7*w3c'
  Identities used:
    Sum_t A[t]*ecum(B)[t] = Sum_t B[t]*S_A[t],  S_A = Sum_{s>t} A[s],
    and S_{dx@K} telescopes to yr@K (so no scan is needed for it);
    EB2*SA9 + w*(SA9-m9h) + v3c'*m9h = (EB2+w)*SA9 + (v3c'-w)*m9h.

Engine plan (v3 trace: DVE 65%, Act 34%, Pool 4% busy — rebalance):
  * PE: 15 fp16 512-col passes/pair; m2/m4/m8 held in PSUM for the terms.
  * Act: 9 PSUM->SBUF bf16 copies/pair (its only job).
  * DVE: bf16 2x tensor_tensor products; p/f2 read PSUM (1x); the three
    STT-accum terms (STT is always 1x, so PSUM operands cost nothing).
  * Pool (GPSIMD, SBUF-only by HW rule): the scan, e1/e2 adds, g2'.

Sharding: pure data parallel, 4 examples per core, packed 2-per-128-partitions.
"""

import numpy as np

import concourse.bass as bass
import concourse.bass_utils as _bu
import concourse.mybir as mybir
import concourse.tile as tile
from concourse.bass_utils import run_bass_kernel_spmd
from bass_rust import ScopedClock


def _patched_drain_and_barrier(self, tick_clock, wait_clock):
    """Split the final drain's sem waits across multiple drain instructions
    (this walrus build rejects >1-2 sync waits per instruction)."""
    drain_inst = self.nc.sync.drain()
    wait_clock.add_sem_waits(drain_inst.ins, ScopedClock({None: tick_clock.global_clock}))
    si = drain_inst.ins.sync_info
    if si is not None and si.on_wait and len(si.on_wait) > 1:
        waits = list(si.on_wait)
        ups = list(si.on_update or [])
        drain_inst.ins.sync_info = mybir.SyncInfo(on_wait=waits[:1], on_update=ups)
        for w in waits[1:]:
            d2 = self.nc.sync.drain()
            d2.ins.sync_info = mybir.SyncInfo(on_wait=[w], on_update=[])

    self.nc.all_engine_barrier()
    popped = self.nc._tile_sem_poison_stack.pop()
    assert popped is self._sem_poison
    self.nc.clear_and_free_semaphores(list(self.sems.allocated().values()))
    self.nc.all_engine_barrier()


tile.TileContext._drain_and_barrier = _patched_drain_and_barrier


def _sanitize_waits(nc, limit=1):
    """Move excess sem waits onto same-engine NOPs inserted just before."""
    import bass_rust

    counter = [0]
    for f in nc.m.functions:
        for blk in f.blocks:
            il = blk.instructions
            i = 0
            while i < len(il):
                inst = il[i]
                si = inst.sync_info
                waits = list(si.on_wait) if (si is not None and si.on_wait) else []
                if len(waits) > limit:
                    keep, extra = waits[:limit], waits[limit:]
                    inst.sync_info = mybir.SyncInfo(
                        on_wait=keep, on_update=list(si.on_update or [])
                    )
                    for j in range(0, len(extra), limit):
                        counter[0] += 1
                        nop = bass_rust.InstNoOp(
                            name=f"waitnop-{counter[0]}", ins=[], outs=[]
                        )
                        nop.engine = inst.engine
                        nop.sync_info = mybir.SyncInfo(
                            on_wait=extra[j : j + limit], on_update=[]
                        )
                        il.insert(i, nop)
                        i += 1
                i += 1
    return counter[0]


B, T, FX = 32, 2048, 63
U = 64
NCORES = 8
BL = B // NCORES
NPAIR = BL // 2

FP = mybir.dt.float32
FH = mybir.dt.float16
BF = mybir.dt.bfloat16
AluOp = mybir.AluOpType

(SK0, SK1, SK2, SK3, SK4, SK5, SK6, SK7, SK8, SK9, SK9H) = range(11)
NSLICE = 11

STREAMS = ["dx", "yb", "yc", "yd2", "yr"]


class PairState:
    pass


def build_nc(sanitize=True):
    nc = bass.Bass("TRN2", target_bir_lowering=False, debug=False)
    st_d = {
        s: nc.dram_tensor(f"{s}p", [NPAIR, 128, T], FH, kind="ExternalInput")
        for s in STREAMS
    }
    d0_d = nc.dram_tensor("d0p", [NPAIR, 128, 1], FH, kind="ExternalInput")
    kst_d = nc.dram_tensor("kst", [NSLICE, 128, 128], FH, kind="ExternalInput")
    out_d = nc.dram_tensor("out", [NPAIR, 128, 1], FP, kind="ExternalOutput")

    with tile.TileContext(nc) as tc:
        with (
            tc.tile_pool(name="pool", bufs=2) as pool,
            tc.tile_pool(name="psum", bufs=2, space="PSUM") as psum,
            tc.tile_pool(name="phold", bufs=2, space="PSUM") as phold,
            tc.tile_pool(name="kpool", bufs=1) as kpool,
        ):
            kst = kpool.tile([128, NSLICE * 128], FH, tag="kst", name="kst")
            nc.sync.dma_start(
                kst[:].rearrange("p (k m) -> p k m", k=NSLICE),
                kst_d.ap().rearrange("k f m -> f k m"),
            )
            zz = kpool.tile([128, T], BF, tag="zz", name="zz")
            nc.gpsimd.memset(zz[:], 0.0)

            def proj(terms, tag, name, p, pl=None):
                """One projection: 2 halves x 2 chunks of 512 cols.
                terms = [(slice_idx, src_tile), ...] accumulated in PSUM."""
                halves = []
                for h in range(2):
                    ps = (pl or psum).tile(
                        [128, 1024], FP, tag=tag, name=f"{name}ps{h}_{p}"
                    )
                    for j in range(2):
                        lo = h * 1024 + j * 512
                        for ti, (si, src) in enumerate(terms):
                            nc.tensor.matmul(
                                ps[:, j * 512 : (j + 1) * 512],
                                kst[:, si * 128 : (si + 1) * 128],
                                src[:, lo : lo + 512],
                                start=(ti == 0),
                                stop=(ti == len(terms) - 1),
                            )
                    halves.append(ps)
                return halves

            def copy_out(halves, name, p, bufs=None, tag=None):
                dst = pool.tile(
                    [128, T], BF, tag=f"c_{tag or name}", name=f"{name}c{p}", bufs=bufs
                )
                for h in range(2):
                    sl = slice(h * 1024, (h + 1) * 1024)
                    nc.scalar.copy(out=dst[:, sl], in_=halves[h][:])
                return dst

            def term(st, zt, mh, c0):
                """acc[:, c0+h] += sum_t zt[:, half h] * mh[h] (PSUM)."""
                for h in range(2):
                    scr = pool.tile(
                        [128, 1024], BF, tag="scr", name=f"scr{st.p}_{c0}_{h}", bufs=2
                    )
                    nc.vector.scalar_tensor_tensor(
                        out=scr[:],
                        in0=zt[:, h * 1024 : (h + 1) * 1024],
                        scalar=1.0,
                        in1=mh[h][:],
                        op0=AluOp.mult,
                        op1=AluOp.mult,
                        accum_out=st.acc[:, c0 + h : c0 + h + 1],
                    )

            def tt_psum(out, sb, ph_halves, op=AluOp.mult):
                """out[128,T] = sb (SBUF) op proj-halves (PSUM), per half."""
                for h in range(2):
                    sl = slice(h * 1024, (h + 1) * 1024)
                    nc.vector.tensor_tensor(
                        out=out[:, sl], in0=sb[:, sl], in1=ph_halves[h][:], op=op
                    )

            def big(name, dt, p, bufs=None):
                return pool.tile([128, T], dt, tag=name, name=f"{name}{p}", bufs=bufs)

            sts = []
            for p in range(NPAIR):
                st = PairState()
                st.p = p
                st.s = {}
                for s in STREAMS:
                    tl = pool.tile([128, T], FH, tag=s, name=f"{s}{p}")
                    for h in range(2):
                        sl = slice(h * 1024, (h + 1) * 1024)
                        nc.sync.dma_start(tl[:, sl], st_d[s][p, :, sl])
                    st.s[s] = tl
                st.d0 = pool.tile([128, 1], FH, tag="d0", name=f"d0{p}")
                nc.sync.dma_start(st.d0[:], d0_d[p])
                st.acc = kpool.tile([128, 8], FP, tag=f"acc{p}", name=f"acc{p}")
                sts.append(st)

            # ---- phase functions (emitted pair-staggered below) ----
            def ph0(st):
                p = st.p
                m7h = proj([(SK7, st.s["dx"])], "ps", "m7", p)
                st.m7c = copy_out(m7h, "m7", p)
                w3h = proj([(SK6, st.s["yb"])], "ps", "w3", p)
                st.pp = big("pp", BF, p, bufs=1)
                tt_psum(st.pp, st.m7c, w3h)
                st.eb2 = big("eb2", BF, p)
                nc.gpsimd.memset(st.eb2[:, 0:1], 0.0)
                nc.vector.tensor_tensor_scan(
                    out=st.eb2[:, 1:T], data0=zz[:, 0 : T - 1],
                    data1=st.pp[:, 0 : T - 1],
                    initial=0.0, op0=AluOp.add, op1=AluOp.add,
                )
                s0 = phold.tile([128, 1024], FP, tag="ph", name=f"s0ps{p}")
                nc.tensor.matmul(
                    s0[:, 0:1],
                    kst[:, SK0 * 128 : (SK0 + 1) * 128],
                    st.d0[:, 0:1],
                    start=True, stop=True,
                )
                st.s0sb = pool.tile([128, 1], FP, tag="s0sb", name=f"s0sb{p}")
                nc.vector.tensor_copy(out=st.s0sb[:], in_=s0[:, 0:1])

            def ph1(st):
                p = st.p
                z1h = proj([(SK1, st.s["yb"])], "ps", "z1", p)
                st.z1c = copy_out(z1h, "z1", p)
                m2h = proj([(SK2, st.s["dx"])], "ph", "m2", p, pl=phold)
                term(st, st.z1c, m2h, 0)

            def ph2(st):
                p = st.p
                w3bh = proj([(SK6, st.s["yc"])], "ps", "w3b", p)
                st.w3bc = copy_out(w3bh, "w3b", p)
                w3ch = proj([(SK6, st.s["yd2"])], "ps", "w3cv", p)
                st.w3cc = copy_out(w3ch, "w3cv", p)
                m9hh = proj([(SK9H, st.s["dx"])], "ps", "m9h", p)
                st.m9hc = copy_out(m9hh, "m9h", p)
                sa9h = proj([(SK9, st.s["yr"]), (SK9H, st.s["dx"])], "ps", "sa9", p)
                st.sa9c = copy_out(sa9h, "sa9", p)
                st.w = big("w", BF, p, bufs=1)
                nc.vector.tensor_tensor(
                    out=st.w[:], in0=st.m7c[:], in1=st.w3bc[:], op=AluOp.mult
                )
                st.v3c = big("v3c", BF, p, bufs=1)
                nc.gpsimd.tensor_tensor(
                    out=st.v3c[:], in0=st.m7c[:], in1=st.w3cc[:], op=AluOp.mult
                )

            def ph3(st):
                p = st.p
                st.e1 = big("e1", BF, p, bufs=1)
                nc.gpsimd.tensor_tensor(
                    out=st.e1[:], in0=st.eb2[:], in1=st.w[:], op=AluOp.add
                )
                st.e2 = big("e2", BF, p, bufs=1)
                nc.gpsimd.tensor_tensor(
                    out=st.e2[:], in0=st.v3c[:], in1=st.w[:], op=AluOp.subtract
                )
                st.g1 = big("g1", BF, p)
                nc.vector.tensor_tensor(
                    out=st.g1[:], in0=st.e1[:], in1=st.sa9c[:], op=AluOp.mult
                )
                st.g2 = big("g2", BF, p, bufs=1)
                nc.gpsimd.tensor_tensor(
                    out=st.g2[:], in0=st.e2[:], in1=st.m9hc[:], op=AluOp.mult
                )
                nc.vector.tensor_tensor(
                    out=st.g1[:], in0=st.g1[:], in1=st.g2[:], op=AluOp.add
                )
                m8h = proj([(SK8, st.s["dx"])], "ph", "m8", p, pl=phold)
                term(st, st.g1, m8h, 2)

            def ph4(st):
                p = st.p
                w2h = proj([(SK3, st.s["yb"])], "ps", "w2", p)
                st.w2c = copy_out(w2h, "w2", p)
                s5h = proj([(SK5, st.s["yr"])], "ps", "s5", p)
                st.s5c = copy_out(s5h, "s5", p)
                st.f1 = big("f1", BF, p)
                nc.vector.tensor_tensor(
                    out=st.f1[:], in0=st.w2c[:], in1=st.s5c[:], op=AluOp.mult
                )
                m5h = proj([(SK5, st.s["dx"])], "ps", "m5", p)
                st.m5c = copy_out(m5h, "m5", p)
                w2bh = proj([(SK3, st.s["yc"])], "ps", "w2b", p)
                st.f2 = big("f2", BF, p, bufs=1)
                tt_psum(st.f2, st.m5c, w2bh)
                nc.gpsimd.tensor_tensor(
                    out=st.f1[:], in0=st.f1[:], in1=st.f2[:], op=AluOp.add
                )
                m4h = proj([(SK4, st.s["dx"])], "ph", "m4", p, pl=phold)
                term(st, st.f1, m4h, 4)

            def ph5(st):
                p = st.p
                red = pool.tile([128, 1], FP, tag="red", name=f"red{p}")
                nc.vector.tensor_reduce(
                    out=red[:], in_=st.acc[:, 0:6], axis=mybir.AxisListType.X,
                    op=AluOp.add,
                )
                outt = pool.tile([128, 1], FP, tag="outt", name=f"outt{p}")
                nc.vector.tensor_tensor(
                    out=outt[:], in0=red[:], in1=st.s0sb[:], op=AluOp.add
                )
                nc.sync.dma_start(out_d[p], outt[:])

            # pair-staggered emission: pair 1 trails pair 0 by one phase so
            # each engine queue always holds ready work from the other pair
            phases = [ph0, ph1, ph2, ph3, ph4, ph5]
            sched = []
            for i in range(len(phases) + 1):
                if i < len(phases):
                    sched.append((phases[i], 0))
                if i >= 1:
                    sched.append((phases[i - 1], 1))
            for fn, p in sched:
                fn(sts[p])
    if sanitize:
        n = _sanitize_waits(nc)
        print(f"[kernel] split {n} excess sem waits onto NOPs")
    return nc


_CACHE = {}


def _get_nc():
    if "nc" not in _CACHE:
        _CACHE["nc"] = build_nc()
    return _CACHE["nc"]


def _bd(k64):
    b = np.zeros((128, 128), dtype=np.float32)
    b[:U, :U] = k64
    b[U:, U:] = k64
    return b


def _marshal(X, kernel):
    Xf = np.ascontiguousarray(X, dtype=np.float32)
    tch = np.arange(T, dtype=np.float32) * (2.0 / (T - 1.0)) - 1.0
    Xa = np.empty((B, T, U), dtype=np.float32)
    Xa[:, :, 0] = tch[None, :]
    Xa[:, :, 1:] = Xf

    ya = np.zeros_like(Xa)
    ya[:, 1:] = Xa[:, :-1] - Xa[:, :1]
    dxa = np.zeros_like(Xa)
    dxa[:, 1:] = Xa[:, 1:] - Xa[:, :-1]
    d0 = Xa[:, -1] - Xa[:, 0]  # [B, U]

    streams = {
        "dx": dxa,
        "yb": ya + 0.5 * dxa,
        "yc": 0.5 * ya + dxa / 6.0,
        "yd2": ya / 3.0 + dxa / 12.0,
        "yr": Xa[:, -1:, :] - Xa,
    }

    def pack(A):  # [B, T, U] -> [NCORES, NPAIR, 128, T]
        return np.ascontiguousarray(
            A.reshape(NCORES, NPAIR, 2, T, U).transpose(0, 1, 2, 4, 3)
        ).reshape(NCORES, NPAIR, 128, T)

    packed = {s: pack(a).astype(np.float16) for s, a in streams.items()}
    d0p = np.ascontiguousarray(d0.reshape(NCORES, NPAIR, 128, 1)).astype(np.float16)

    kf = np.asarray(kernel, dtype=np.float32)
    K = [kf[:, i, :] for i in range(10)]
    slices = [K[0], K[1], K[2], K[3], K[4], K[5], K[6], K[7], K[8], K[9], 0.5 * K[9]]
    kst16 = np.stack([_bd(s) for s in slices]).astype(np.float16)
    return packed, d0p, kst16


def run(X, kernel, trace=False):
    nc = _get_nc()
    packed, d0p, kst = _marshal(X, kernel)
    in_maps = [
        {
            **{f"{s}p": packed[s][c] for s in STREAMS},
            "d0p": d0p[c],
            "kst": kst,
        }
        for c in range(NCORES)
    ]
    res = run_bass_kernel_spmd(nc, in_maps, list(range(NCORES)), trace=trace)
    out = np.stack([r["out"] for r in res.results])
    out = out.reshape(NCORES, NPAIR, 2, U).reshape(B, U)
    return out, res


def kernel(X, kernel):
    out, _ = run(X, kernel)
    return out
